# revision 1
# baseline (speedup 1.0000x reference)
"""Multi-head attention (B=4, N=2048, D=512, H=8, Dh=64) on 8 TRN2 NeuronCores.

Sharding: core c handles batch b=c//2, query-row half r=c%2 (1024 rows), all 8
heads. k/v are computed for the full 2048-row sequence on each core (duplicated
across the pair of cores sharing a batch), so there is no cross-core
communication; the host gather is pure placement.

Device kernel (per core), all matmuls float32r:
  phase A: qkvT projection from host-pre-transposed xT (f32r, DMA-direct)
  phase B: per head: scoresT[j,i] = kT.T @ qT; PT = exp(scoresT/8) (ACT, f32r);
           outT[d,i] (+ Z row via ones column in [v|1] lhsT) = [v|1].T @ PT;
           normalize via reciprocal + partition-broadcast DMA + DVE multiply
  phase C: y[i,e] = outTn.T @ w_out + b_out (bias as K=1 matmul row)
"""

import numpy as np

import concourse.bass as bass
import concourse.tile as tile
from concourse import mybir
from concourse.bass_utils import run_bass_kernel_spmd
from concourse.tile_rust import add_dep_helper
import concourse.tile_sem_assignment as _tsa

# ONE HWDGE DMA semaphore lane: the kernel-tail drain waits on every engine
# + every DMA lane used, and walrus accepts very few sync waits per
# instruction. All HWDGE DMAs share one ring anyway, so one lane only
# coarsens completion tracking, not DMA throughput.
_tsa.NUM_HWDGE_SEMS = 1

F32 = mybir.dt.float32
F32R = mybir.dt.float32r
EXP = mybir.ActivationFunctionType.Exp

B, N, D = 4, 2048, 512
H, DH = 8, 64
SCALE = DH ** -0.5
ROWS = N // 2          # query rows per core
NC_COUNT = 8
CCH = D // 128         # contraction chunks over D (4)
JT = N // 128          # j tiles (16)
VW = DH + 1            # v columns per head incl. ones column (65)


def _round_f32r(x):
    """Round fp32 to the f32r grid (11-bit mantissa), round-to-nearest-even."""
    u = np.ascontiguousarray(x).view(np.uint32)
    bias = ((u >> 12) & 1).astype(np.uint32) + np.uint32(0x7FF)
    return ((u + bias) & np.uint32(0xFFFFF000)).view(np.float32)


def _build_nc():
    nc = bass.Bass("TRN2", target_bir_lowering=False, debug=False,
                   num_devices=NC_COUNT)
    xT = nc.dram_tensor("xT", [D, N], F32R, kind="ExternalInput").ap()
    wqkv = nc.dram_tensor("wqkv", [D, 3 * D], F32R, kind="ExternalInput").ap()
    wout = nc.dram_tensor("wout", [D, D], F32R, kind="ExternalInput").ap()
    bout = nc.dram_tensor("bout", [1, D], F32R, kind="ExternalInput").ap()
    Y = nc.dram_tensor("Y", [ROWS, D], F32, kind="ExternalOutput").ap()

    with tile.TileContext(nc) as tc:
        _emit(nc, tc, xT, wqkv, wout, bout, Y)
    _strip_act_self_waits(nc)
    _strip_dma_self_lane_waits(nc)
    _strip_far_mm_pe_waits(nc)
    _fix_tail_drain(nc)
    return nc


def _strip_far_mm_pe_waits(nc):
    """Drop far-distance PE waits from 2-wait matmuls.

    The wait assigner sometimes covers an ancient (already transitively
    satisfied) dependency with a PE wait instead of eliding it. A genuine
    PSUM WAW pipeline guard targets the immediately preceding matmuls; a PE
    wait >=4 instructions back on a matmul that also carries a real data
    wait is one of these redundant covers."""
    for f in nc.m.functions:
        for blk in f.blocks:
            pe_idx = 0
            for inst in blk.instructions:
                if str(getattr(inst, "engine", "")) != "EngineType.PE":
                    continue
                pe_idx += 1
                if type(inst).__name__ != "InstMatmult":
                    continue
                si = inst.sync_info
                if si is None or len(si.on_wait) <= 1:
                    continue
                keep = [w for w in si.on_wait
                        if not (w.ant_name.startswith("PE")
                                and pe_idx - w.wait_value >= 4)]
                assert len(keep) <= 1, (
                    f"{inst.name}: waits {[(w.ant_name, w.wait_value) for w in si.on_wait]} pe_idx={pe_idx}")
                inst.sync_info = mybir.SyncInfo(on_wait=keep,
                                                on_update=list(si.on_update))


def _fix_tail_drain(nc):
    """Give the kernel-tail drain a single deterministic wait: the final
    DMAHW0 tick. The three tail consolidation DMAs executed their engine
    waits (ACT/PE/DVE final ticks) on the SP sequencer before pushing their
    descriptors, and the single HWDGE ring drains FIFO per SDMA engine, so
    "last DMA's increments landed" implies every engine finished and every
    earlier DMA (including all Y writes) completed."""
    n_dma = 0
    lane_wait = None
    last_dma_pos = -1
    seq = []
    for f in nc.m.functions:
        for blk in f.blocks:
            for inst in blk.instructions:
                seq.append(inst)
                nm = type(inst).__name__
                if nm == "InstDMACopy":
                    n_dma += 1
                    last_dma_pos = len(seq) - 1
                    si = inst.sync_info
                    if si:
                        for u in si.on_update:
                            if u.ant_name.startswith("DMAHW0"):
                                lane_wait = u
    tail_drain = None
    for inst in seq[last_dma_pos + 1:]:
        if (type(inst).__name__ == "InstDrain"
                and str(inst.engine) == "EngineType.SP"):
            si = inst.sync_info
            waits = si.on_wait if si else []
            if not any(w.ant_name.startswith("barrier") for w in waits):
                tail_drain = inst
                break
    assert tail_drain is not None and lane_wait is not None
    w = mybir.SyncWait(sync_type="semaphore", id=lane_wait.id,
                       ant_name=lane_wait.ant_name, wait_mode="sem-ge-imm",
                       wait_value=16 * n_dma, wait_reg=None)
    si = tail_drain.sync_info
    tail_drain.sync_info = mybir.SyncInfo(
        on_wait=[w], on_update=list(si.on_update) if si else [])


def _strip_dma_self_lane_waits(nc):
    """Drop DMA-lane (DMAHW*) waits from DMACopy instructions.

    All HWDGE DMAs here issue via nc.sync onto ONE ring (qSPDynamicHW). Each
    SDMA engine drains that ring FIFO, so any earlier-issued DMA's sem
    increments land before a later DMA's: DMA-to-DMA ordering is implicit in
    the ring and the explicit lane waits are redundant. DMACopy accepts only
    one sync wait, which the (engine) data dependency needs."""
    for f in nc.m.functions:
        for blk in f.blocks:
            for inst in blk.instructions:
                if type(inst).__name__ != "InstDMACopy":
                    continue
                si = inst.sync_info
                if si is None or len(si.on_wait) <= 1:
                    continue
                keep = [w for w in si.on_wait
                        if not w.ant_name.startswith("DMAHW")]
                assert len(keep) <= 1, (
                    f"{inst.name}: waits {[(w.ant_name, w.wait_value) for w in si.on_wait]}")
                inst.sync_info = mybir.SyncInfo(on_wait=keep,
                                                on_update=list(si.on_update))


def _strip_act_self_waits(nc):
    """Drop self-engine-sem waits from ACT and DVE instructions.

    The ACT and DVE engines execute instructions strictly in order and drain
    each op before the next (no cross-instruction memory overlap), so a wait
    on the instruction's own engine semaphore is pure ordering — implicit in
    program order — and redundant. Walrus accepts only ONE sync wait per
    instruction in this build, so these must go."""
    self_sem = {"Activation": "Activation", "DVE": "DVE"}
    for f in nc.m.functions:
        for blk in f.blocks:
            for inst in blk.instructions:
                eng = str(getattr(inst, "engine", ""))
                pref = None
                if eng == "EngineType.Activation":
                    pref = "Activation"
                elif eng == "EngineType.DVE":
                    pref = "DVE"
                if pref is None:
                    continue
                si = inst.sync_info
                if si is None or len(si.on_wait) <= 1:
                    continue
                keep = [w for w in si.on_wait
                        if not w.ant_name.startswith(pref)]
                assert len(keep) <= 1, (
                    f"{inst.name}: unexpected waits {[(w.ant_name, w.wait_value) for w in si.on_wait]}")
                inst.sync_info = mybir.SyncInfo(on_wait=keep,
                                                on_update=list(si.on_update))


class _RotSlots:
    """Track writer/reader instructions per rotating pool slot so a bf16
    dummy matmul can absorb all outstanding ticks before a float32r matmul
    reopens the slot (f32r matmuls tolerate only ONE sync wait)."""

    def __init__(self, bufs):
        self.slots = [None] * bufs
        self.i = 0

    def open(self):
        """Return dep instructions to absorb; start tracking the new use."""
        rec = self.slots[self.i % len(self.slots)]
        deps = []
        if rec is not None:
            deps = [d for d in ([rec["w"]] + rec["r"]) if d is not None]
        new = {"w": None, "r": []}
        self.slots[self.i % len(self.slots)] = new
        self.i += 1
        return deps, new


def _emit(nc, tc, xT, wqkv, wout, bout, Y):
    from contextlib import ExitStack
    with ExitStack() as ctx:
        P = ctx.enter_context(tc.tile_pool(name="persist", bufs=1))
        pt_pool = ctx.enter_context(tc.tile_pool(name="pt", bufs=4))
        rz_pool = ctx.enter_context(tc.tile_pool(name="rz", bufs=2))
        bc_pool = ctx.enter_context(tc.tile_pool(name="bc", bufs=2))
        y_pool = ctx.enter_context(tc.tile_pool(name="ysb", bufs=3))
        scr = ctx.enter_context(tc.tile_pool(name="scrps", bufs=1, space="PSUM"))

        # ---- input DMAs (f32r tiles, host pre-rounded), order matters for
        # ---- the lane-covering dummy matmuls below (3 lanes, round-robin)
        xt = [P.tile([128, N], F32R, tag=f"xt{c}", name=f"xt{c}") for c in range(CCH)]
        wq = [P.tile([128, 3 * D], F32R, tag=f"wq{c}", name=f"wq{c}") for c in range(CCH)]
        wo = [P.tile([128, D], F32R, tag=f"wo{c}", name=f"wo{c}") for c in range(CCH)]
        bo = P.tile([1, D], F32R, tag="bo")
        dmas = []
        for c in range(CCH):
            dmas.append((nc.sync.dma_start(xt[c][:], xT[c * 128:(c + 1) * 128, :]), xt[c]))
        for c in range(CCH):
            dmas.append((nc.sync.dma_start(wq[c][:], wqkv[c * 128:(c + 1) * 128, :]), wq[c]))
        for c in range(CCH):
            dmas.append((nc.sync.dma_start(wo[c][:], wout[c * 128:(c + 1) * 128, :]), wo[c]))
        dmas.append((nc.sync.dma_start(bo[:], bout), bo))

        # DVE-produced constants (f32r memset fails the ISA value-type
        # check, so round-copy from an fp32 ones source instead)
        onesrc = P.tile([128, 128], F32, tag="onesrc")
        nc.vector.memset(onesrc[:], 1.0)
        ones_b = P.tile([1, 128], F32R, tag="onesb")
        nc.vector.tensor_copy(ones_b[:], onesrc[0:1, :])
        bconst = P.tile([1, 2], mybir.dt.bfloat16, tag="bconst")
        nc.vector.memset(bconst[:], 1.0)
        scr_ps = scr.tile([1, 512], F32, tag="scr")

        def absorb(deps):
            """Chain of tiny bf16 dummy matmuls, ONE per dependency (every
            matmul instruction tolerates only a single sync wait), that sync
            the in-order PE on all given instructions so the next f32r matmul
            needs <=1 wait. Returns the last dummy; the group's first real
            matmul must take a nosync dep on it (scheduler hoist guard)."""
            deps = [d for d in deps if d is not None]
            if not deps:
                return None
            seen = set()
            dmm = None
            for d in deps:
                if d.ins.name in seen:
                    continue
                seen.add(d.ins.name)
                prev = dmm
                dmm = nc.tensor.matmul(scr_ps[0:1, 0:2], bconst[0:1, 0:1],
                                       bconst[0:1, 0:2], start=True, stop=True)
                add_dep_helper(dmm.ins, d.ins, reason="absorb tick for f32r matmul")
                if prev is not None:
                    add_dep_helper(dmm.ins, prev.ins,
                                   reason="absorber chain order")
            return dmm

        def order_after(mm, dmm):
            if dmm is not None:
                add_dep_helper(mm.ins, dmm.ins, sync=False,
                               reason="keep f32r matmul after its absorber")

        dve_scr = P.tile([1, 4], F32, tag="dvescr")

        def dve_absorb(deps):
            """Chain of tiny DVE memsets, one per dep, absorbing ticks the
            next DVE instruction would otherwise have to wait on itself."""
            deps = [d for d in deps if d is not None]
            last = None
            for d in deps:
                dm = nc.vector.memset(dve_scr[0:1, 0:1], 0.0)
                add_dep_helper(dm.ins, d.ins, reason="DVE absorbs tick")
                if last is not None:
                    add_dep_helper(dm.ins, last.ins, sync=False,
                                   reason="DVE absorber chain")
                last = dm
            return last

        # persistent activation tensors
        qt = [P.tile([128, ROWS], F32R, tag=f"qt{t}", name=f"qt{t}") for t in range(4)]
        kt = [P.tile([128, N], F32R, tag=f"kt{t}", name=f"kt{t}") for t in range(4)]
        vt = [P.tile([128, H * VW], mybir.dt.bfloat16, tag=f"vt{j}", name=f"vt{j}") for j in range(JT)]
        on = [P.tile([128, ROWS], F32R, tag=f"on{t}", name=f"on{t}") for t in range(4)]

        with tc.tile_pool(name="qkvps", bufs=2, space="PSUM") as qps:
            qkv_rot = _RotSlots(2)
            last_copy = {"qk": None}
            vt_copies = []

            def qkv_group(lhs_fn, extra_deps=()):
                deps, rec = qkv_rot.open()
                # absorb only the DVE reader ticks; the PE-self WAW wait can
                # ride on the real matmul (one wait is legal)
                deps = [d for d in deps
                        if str(d.ins.engine) != "EngineType.PE"]
                dmm = absorb(list(deps) + list(extra_deps))
                ps = qps.tile([128, 512], F32, tag="qkv", name="qkv_ps")
                for c in range(CCH):
                    rec["w"] = nc.tensor.matmul(ps[:], *lhs_fn(c),
                                                start=(c == 0),
                                                stop=(c == CCH - 1))
                    if c == 0:
                        order_after(rec["w"], dmm)
                return ps, rec

            # ---- phase A: v, kT, qT (qT last so its DVE copies carry the
            # ---- highest DVE ticks when phase B starts)
            for j in range(JT):
                extra = dmas[7:8] if j == 0 else ()  # covers all xt+wq DMAs (1 lane)
                ps, rec = qkv_group(
                    lambda c, j=j: (xt[c][:, j * 128:(j + 1) * 128],
                                    wq[c][:, 2 * D:3 * D]),
                    extra_deps=[d for d, _ in extra])
                vv = vt[j][:].rearrange("p (h e) -> p h e", e=VW)
                cp = nc.vector.tensor_copy(vv[:, :, 0:DH],
                                           ps[:].rearrange("p (h e) -> p h e", e=DH))
                rec["r"].append(cp)
                cpo = nc.vector.tensor_copy(vv[:, :, DH:VW],
                                            onesrc[:, 0:H].unsqueeze(2))
                vt_copies.append([cp, cpo])
            last_kq_copy = {}

            def emit_kq(t, group_fn):
                """kT and qT projection groups for head-pair tile t."""
                for i2k in range(N // 512):
                    ps, rec = group_fn(
                        lambda c, t=t, i2=i2k: (
                            wq[c][:, D + t * 128:D + (t + 1) * 128],
                            xt[c][:, i2 * 512:(i2 + 1) * 512]))
                    cp = nc.vector.tensor_copy(
                        kt[t][:, i2k * 512:(i2k + 1) * 512], ps[:])
                    rec["r"].append(cp)
                for i2k in range(ROWS // 512):
                    ps, rec = group_fn(
                        lambda c, t=t, i2=i2k: (
                            wq[c][:, t * 128:(t + 1) * 128],
                            xt[c][:, i2 * 512:(i2 + 1) * 512]))
                    cp = nc.vector.tensor_copy(
                        qt[t][:, i2k * 512:(i2k + 1) * 512], ps[:])
                    rec["r"].append(cp)
                    last_kq_copy[t] = cp

            for t in range(4):
                emit_kq(t, qkv_group)

        with (
            tc.tile_pool(name="scps", bufs=2, space="PSUM") as sps,
            tc.tile_pool(name="ops", bufs=2, space="PSUM") as ops,
            tc.tile_pool(name="zbps", bufs=1, space="PSUM") as zbp,
        ):
            # ---- phase B: attention, one sweep per (head-pair, i-chunk).
            # The even head lives on partitions 0-63 of its qt/kt tile and
            # the odd head on 64-127, so the two K=64 score matmuls go to
            # disjoint PE row groups and run CONCURRENTLY (tile_position
            # auto-derives from base_partition). exp covers both heads in
            # one [128,1024] pass. Softmax normalization: Z row -> SBUF,
            # K=1 matmul broadcasts it across 64 partitions, DVE divide.
            IC = ROWS // 512  # i chunks of 512 (2)
            sc_rot = _RotSlots(2)
            o_rot = _RotSlots(2)
            zb_rot = _RotSlots(1)
            last_exp = [None]
            norm_insts = {}   # (h, i2) -> final normalize instruction
            # ACT absorber: the first exp touches PSUM banks released by the
            # qkv pool, which adds a DVE release dep on top of its PE wait;
            # Activation instructions also tolerate only one sync wait.
            act_scr = P.tile([1, 4], F32, tag="actscr")
            act_dummy = nc.scalar.copy(act_scr[0:1, 0:2], bconst[0:1, 0:2])
            add_dep_helper(act_dummy.ins, last_kq_copy[H // 2 - 1].ins,
                           reason="ACT absorbs phase-A DVE tick")
            first_exp_done = False
            first_sc_done = False
            for hp in range(H // 2):
                for i2 in range(IC):
                    isl = slice(i2 * 512, (i2 + 1) * 512)
                    o_ps, o_recs, o_deps = [], [], []
                    for par in range(2):
                        deps, rec = o_rot.open()
                        # the j=0 AV matmul also depends on the vt[0] copies
                        o_deps.append(list(deps) + vt_copies[0])
                        o_ps.append(ops.tile([65, 512], F32, tag="o",
                                             name="o_ps"))
                        o_recs.append(rec)
                    # software pipeline: emit scores(j)+exp(j), THEN the AV
                    # pair for j-1, so the PE never blocks the next exp's
                    # input behind AV matmuls that wait on the current exp.
                    def emit_av(pt, j):
                        for par in range(2):
                            h = 2 * hp + par
                            # sweep-open absorbers emitted HERE (late) so the
                            # PE doesn't stall on them before the first
                            # scores/exp of the sweep
                            dmm = absorb(o_deps[par]) if j == 0 else None
                            o_recs[par]["w"] = nc.tensor.matmul(
                                o_ps[par][:],
                                vt[j][:, h * VW:(h + 1) * VW],
                                pt[:, par * 512:(par + 1) * 512],
                                start=(j == 0), stop=(j == JT - 1))
                            if j == 0:
                                order_after(o_recs[par]["w"], dmm)
                    pending = None
                    for j in range(JT):
                        deps, sc_rec = sc_rot.open()
                        # steady state: the slot's WAR (exp j-2) is covered by
                        # the previous AV's ACT wait and the WAW rides as the
                        # matmul's single PE-self wait; the first scores
                        # matmul of each head-pair needs DVE coverage of its
                        # (possibly freshly interleaved) kT/qT copies
                        if j == 0 and i2 == 0:
                            sc_dmm = absorb([last_kq_copy[hp]])
                        else:
                            sc_dmm = None
                        sc = sps.tile([128, 1024], F32, tag="sc", name="sc_ps")
                        for par in range(2):
                            rg = slice(par * 64, (par + 1) * 64)
                            mm = nc.tensor.matmul(
                                sc[:, par * 512:(par + 1) * 512],
                                kt[hp][rg, j * 128:(j + 1) * 128],
                                qt[hp][rg, isl],
                                start=True, stop=True)
                            sc_rec["w"] = mm
                            if par == 0:
                                order_after(mm, sc_dmm)
                        pt = pt_pool.tile([128, 1024], mybir.dt.bfloat16,
                                          tag="pt", name="pt_sb")
                        ex = nc.scalar.activation(pt[:], sc[:], EXP,
                                                  scale=SCALE)
                        last_exp[0] = ex
                        if not first_exp_done:
                            first_exp_done = True
                            add_dep_helper(ex.ins, act_dummy.ins,
                                           reason="first exp after ACT absorber")
                        sc_rec["r"].append(ex)
                        if pending is not None:
                            emit_av(*pending)
                        pending = (pt, j)
                    emit_av(*pending)
                    # pass 1: free both PSUM slots with fast copies so the
                    # next sweep's AV matmuls can reopen them immediately
                    z_sbs, u_sbs = [], []
                    for par in range(2):
                        z_sb = rz_pool.tile([1, 512], F32, tag="zsb",
                                            name="z_sb")
                        zc = nc.vector.tensor_copy(z_sb[:], o_ps[par][64:65, :])
                        u_sb = bc_pool.tile([64, 512], F32, tag="bc",
                                            name="u_sb")
                        uc = nc.vector.tensor_copy(u_sb[:], o_ps[par][0:64, :])
                        o_recs[par]["r"].extend([zc, uc])
                        z_sbs.append(z_sb)
                        u_sbs.append(u_sb)
                    # pass 2 (off the slot critical path): approx reciprocal
                    # (~18 correct bits, 5x faster than InstReciprocal),
                    # round to f32r, PE-broadcast, multiply
                    for par in range(2):
                        h = 2 * hp + par
                        rz = rz_pool.tile([1, 512], F32R, tag="rz",
                                          name="rz_sb")
                        with nc.allow_low_precision(
                                reason="f32r 1/Z: 2^-12 relative rounding is fine"):
                            rc = nc.vector.reciprocal(rz[:], z_sbs[par][:])
                        zdeps, zrec = zb_rot.open()
                        zb_dmm = absorb([d for d in zdeps
                                         if d is not None and
                                         str(d.ins.engine) == "EngineType.PE"])
                        zb = zbp.tile([64, 512], F32, tag="zb", name="zb_ps")
                        zmm = nc.tensor.matmul(zb[:], ones_b[0:1, 0:64],
                                               rz[:], start=True, stop=True)
                        order_after(zmm, zb_dmm)
                        zrec["w"] = zmm
                        ni = nc.vector.tensor_mul(
                            on[hp][par * 64:(par + 1) * 64, isl],
                            u_sbs[par][:], zb[:])
                        zrec["r"].append(ni)
                        norm_insts[(h, i2)] = ni

            # ---- phase C: output projection + bias
            y_dmas = []
            last_insts = {"pe": None, "dve": None, "act": last_exp[0]}
            for ic in range(ROWS // 128):
                deps, rec = o_rot.open()
                if ic == 0:
                    deps = list(deps) + [dmas[-1][0]] + \
                        [norm_insts[(H - 1, 0)], norm_insts[(H - 1, 1)]]
                p_dmm = absorb(deps)
                yp = ops.tile([128, 512], F32, tag="o", name="yp_ps")
                for fc in range(4):
                    mm = nc.tensor.matmul(yp[:], on[fc][:, ic * 128:(ic + 1) * 128],
                                          wo[fc][:], start=(fc == 0), stop=False)
                    if fc == 0:
                        order_after(mm, p_dmm)
                rec["w"] = nc.tensor.matmul(yp[:], ones_b[:], bo[:],
                                            start=False, stop=True)
                ysb = y_pool.tile([128, 512], F32, tag="ysb", name="ysb_sb")
                dvd = dve_absorb([y_dmas[ic - 3]] if ic >= 3 else [])
                cp = nc.vector.tensor_copy(ysb[:], yp[:])
                if dvd is not None:
                    add_dep_helper(cp.ins, dvd.ins, sync=False,
                                   reason="copy after DVE absorber")
                rec["r"].append(cp)
                y_dmas.append(nc.sync.dma_start(Y[ic * 128:(ic + 1) * 128, :], ysb[:]))
                last_insts["pe"] = rec["w"]
                last_insts["dve"] = cp

            # ---- tail consolidation: tiny chained DMAs make the SP
            # ---- sequencer observe each engine's final tick, so the
            # ---- kernel-tail drain needs only its single DMA-lane wait.
            tail_scr = P.tile([1, 4], F32, tag="tailscr")
            nc.vector.memset(tail_scr[:], 0.0)
            prev = y_dmas[-1]  # keep the chain after all output DMAs
            for key in ("act", "pe", "dve"):
                d = last_insts[key]
                if d is None:
                    continue
                td = nc.sync.dma_start(tail_scr[0:1, 1:2], tail_scr[0:1, 0:1])
                add_dep_helper(td.ins, d.ins, reason=f"tail consolidate {key}")
                if prev is not None:
                    add_dep_helper(td.ins, prev.ins, sync=False,
                                   reason="tail chain order")
                prev = td


_NC_CACHE = {}
LAST_EXEC_NS = None


def _get_nc():
    if "nc" not in _NC_CACHE:
        _NC_CACHE["nc"] = _build_nc()
    return _NC_CACHE["nc"]


def kernel(x, w_qkv, w_out, b_out, trace=False):
    global LAST_EXEC_NS
    x = np.asarray(x, dtype=np.float32)
    w_qkv_r = _round_f32r(np.asarray(w_qkv, dtype=np.float32))
    w_out_r = _round_f32r(np.asarray(w_out, dtype=np.float32))
    b_out_r = _round_f32r(np.asarray(b_out, dtype=np.float32).reshape(1, D))

    in_maps = []
    for c in range(NC_COUNT):
        b, r = c // 2, c % 2
        xTb = x[b].T  # [D, N]
        own = xTb[:, r * ROWS:(r + 1) * ROWS]
        other = xTb[:, (1 - r) * ROWS:(2 - r) * ROWS]
        xT_rot = _round_f32r(np.ascontiguousarray(
            np.concatenate([own, other], axis=1)))
        in_maps.append({"xT": xT_rot, "wqkv": w_qkv_r, "wout": w_out_r,
                        "bout": b_out_r})

    nc = _get_nc()
    res = run_bass_kernel_spmd(nc, in_maps, list(range(NC_COUNT)), trace=trace)
    LAST_EXEC_NS = res.exec_time_ns

    out = np.empty((B, N, D), dtype=np.float32)
    for c in range(NC_COUNT):
        b, r = c // 2, c % 2
        out[b, r * ROWS:(r + 1) * ROWS, :] = res.results[c]["Y"]
    return out

